# revision 6
# baseline (speedup 1.0000x reference)
"""Trainium2 Bass kernel for nn_AdaptiveFusion.

Math (per batch b):
  q  = x @ Wq.T + bq                         (L,H)
  kv = g @ Wkv.T + bkv ; k,v = split         (Lg,H) each
  p  = softmax(q @ k.T / sqrt(H))            (L,Lg)
  gc = p @ v                                 (L,H)
  g1 = sigmoid(x @ W1x.T + gc @ W1y.T + bg1) (L,H)   [k-independent]
  h1 = gc + g1*(x - gc)                      (L,H)
  A  = h1 @ W2x.T                            (L,H)
  C  = s @ W2y.T + bg2                       (K,H)
  out[l,k,o] = s[k,o] + sigmoid(A[l,o]+C[k,o]) * (h1[l,o]-s[k,o])

Sharding: data-parallel over B (8 batches -> 8 cores), weights replicated,
no collectives.

v3 structure:
  - all weights fp8-e3m4 (x32 prescale, descaled free via ACT scale)
  - PE warm-up dummy matmuls from t=0 so the real stream runs at full clock
  - need-ordered single-queue input DMA; w2x/w2y interleaved per-oc-pair
  - full-L chain (minimal LDWEIGHTS count), then A/C for all oc hoisted
  - combine per oc:
      k <  KF : sig_k = Sigmoid(A + C_k) per-k on ACT (bias trick)
      k >= KF : arg = A + C_rep via one DVE TT, one batched ACT sigmoid
      m_k     = (h1 - s_k) * sig_k  -- per-k DVE scalar_tensor_tensor
                (fuses the old sub+mul passes; scalar s_k comes from sT)
      out     = m + s_rep -- batched DVE TT per k-half; last GA k's of each
                oc instead take a GPSIMD per-k tensor_scalar add
Output DMA in [OC, 128, K, L] layout (the SBUF tile layout -> fully linear
descriptors); the host permutes back.
"""

import os
import sys

import numpy as np

if "/opt/trn_rl_repo" not in sys.path:
    sys.path.insert(0, "/opt/trn_rl_repo")

import ml_dtypes

BF16 = ml_dtypes.bfloat16
F8E3 = ml_dtypes.float8_e3m4

B, L, K, Lg, H = 8, 256, 32, 128, 768
HC = H // 128  # h-chunks
OC = H // 128  # o-chunks
LR = 8         # dummy-l width of the host-replicated s
KF = 16        # k's handled by fused per-k ACT sigmoid; rest via DVE arg
KS = K - KF
GA = 6         # trailing k's per oc whose final add runs on GPSIMD
W8SCALE = 32.0
DUMN = 24      # PE warm-up dummy matmuls

_CACHE = {}

last_exec_time_ns = None
last_profile = None


def _build():
    import concourse.bacc as bacc
    import concourse.bass as bass
    import concourse.mybir as mybir
    import concourse.tile as tile

    f32 = mybir.dt.float32
    bf16 = mybir.dt.bfloat16
    fp8 = mybir.dt.float8e3
    AF = mybir.ActivationFunctionType
    OP = mybir.AluOpType

    nc = bacc.Bacc(None, target_bir_lowering=False, debug=False)

    u8 = mybir.dt.uint8
    # acts layout per partition: bstack f32 (120B) | xT (3072B) | gT (1536B)
    #                            | sT (384B) | ident (256B)
    ABYTES = 120 + 3072 + 1536 + 384 + 256
    acts = nc.declare_dram_parameter("acts", [128, ABYTES], u8, isOutput=False)
    srep = nc.declare_dram_parameter("srep", [128, OC, K, LR], bf16, isOutput=False)
    wq_d = nc.declare_dram_parameter("wq", [128, HC, H], fp8, isOutput=False)
    wk_d = nc.declare_dram_parameter("wk", [128, HC, H], fp8, isOutput=False)
    wv_d = nc.declare_dram_parameter("wv", [128, HC, H], fp8, isOutput=False)
    w1x_d = nc.declare_dram_parameter("w1x", [128, HC, H], fp8, isOutput=False)
    w1y_d = nc.declare_dram_parameter("w1y", [128, HC, H], fp8, isOutput=False)
    # w2x/w2y interleaved per oc-pair so the first combine unblocks early
    w2xy_d = [nc.declare_dram_parameter(f"w2xy{i}", [128, 2, HC, 256], fp8,
                                        isOutput=False) for i in range(3)]
    out_d = nc.declare_dram_parameter("out", [OC, 128, K, L], bf16, isOutput=True)

    inv_w8 = 1.0 / W8SCALE
    inv_sqrt_h = 1.0 / float(np.sqrt(H)) * inv_w8

    with tile.TileContext(nc) as tc:
        with (
            tc.tile_pool(name="wpool", bufs=1) as wpool,
            tc.tile_pool(name="apool", bufs=1) as apool,
            tc.tile_pool(name="ppool", bufs=3, space=bass.MemorySpace.PSUM) as ppool,
            tc.tile_pool(name="vpool", bufs=1, space=bass.MemorySpace.PSUM) as vpool,
            tc.tile_pool(name="atp", bufs=2, space=bass.MemorySpace.PSUM) as atp,
            tc.tile_pool(name="wmp", bufs=1, space=bass.MemorySpace.PSUM) as wmp,
            tc.tile_pool(name="spool", bufs=3) as spool,
            tc.tile_pool(name="opool", bufs=2) as opool,
            tc.tile_pool(name="dpool", bufs=2) as dpool,
        ):
            # ---- PE warm-up: dummy matmuls with no external deps ----
            dum = spool.tile([128, 256], bf16, tag="dum")
            nc.vector.memset(dum[:], 0)
            for i in range(DUMN):
                psd = wmp.tile([128, 256], f32, tag="psd")
                nc.tensor.matmul(psd[:], dum[:, 0:128], dum[:], start=True,
                                 stop=True)

            # ---- input DMAs: one queue (sync), strict need order ----
            acts_s = apool.tile([128, ABYTES], u8)
            nc.sync.dma_start(acts_s[:], acts[:])
            bst_s = acts_s[:, 0:120].bitcast(f32)
            xT_s = acts_s[:, 120:3192].bitcast(bf16).rearrange(
                "p (c l) -> p c l", c=HC)
            gT_s = acts_s[:, 3192:4728].bitcast(bf16).rearrange(
                "p (c l) -> p c l", c=HC)
            sT_s = acts_s[:, 4728:5112].bitcast(bf16).rearrange(
                "p (c l) -> p c l", c=HC)
            id_s = acts_s[:, 5112:5368].bitcast(bf16)

            def wload(name, src):
                t = wpool.tile([128, HC, H], fp8, tag=name)
                nc.sync.dma_start(t[:], src[:])
                return t

            wk_s = wload("wk", wk_d)
            wq_s = wload("wq", wq_d)
            wv_s = wload("wv", wv_d)
            w1y_s = wload("w1y", w1y_d)
            w1x_s = wload("w1x", w1x_d)
            w2xy_s = []
            for i in range(3):
                t = wpool.tile([128, 2, HC, 256], fp8, tag=f"w2xy{i}")
                nc.sync.dma_start(t[:], w2xy_d[i][:])
                w2xy_s.append(t)
            srep_s = apool.tile([128, OC, K, LR], bf16)
            nc.sync.dma_start(srep_s[:], srep[:])

            def w2x(oc):  # [128(h), HC, 128] fp8 slice for output chunk oc
                return w2xy_s[oc // 2][:, 0, :, (oc % 2) * 128:(oc % 2) * 128 + 128]

            def w2y(oc):
                return w2xy_s[oc // 2][:, 1, :, (oc % 2) * 128:(oc % 2) * 128 + 128]

            bqs_s, bk_s, bv_s = bst_s[:, 0:OC], bst_s[:, OC:2 * OC], bst_s[:, 2 * OC:3 * OC]
            bg1_s, bg2_s = bst_s[:, 3 * OC:4 * OC], bst_s[:, 4 * OC:5 * OC]

            # warm only the Exp table; the Sigmoid set loads once later
            scratch = spool.tile([128, 2], f32, tag="warm")
            nc.scalar.activation(scratch[:, 0:1], bst_s[:, 0:1], AF.Exp)

            # f32 copy of s^T for GPSIMD tensor_scalar adds
            sF_s = apool.tile([128, OC, K], f32)
            nc.vector.tensor_copy(sF_s[:], sT_s[:])

            # ---- k^T[o,m] ----
            kT_s = apool.tile([128, OC, Lg], bf16)
            for oc in range(OC):
                ps = ppool.tile([128, Lg], f32, tag="ps")
                for hc in range(HC):
                    nc.tensor.matmul(
                        ps[:], wk_s[:, hc, oc * 128:(oc + 1) * 128], gT_s[:, hc, :],
                        start=(hc == 0), stop=(hc == HC - 1))
                nc.scalar.activation(kT_s[:, oc, :], ps[:], AF.Identity,
                                     bias=bk_s[:, oc:oc + 1], scale=inv_w8)

            # ---- q^T[o,l] ----
            qT_s = apool.tile([128, OC, L], bf16)
            for oc in range(OC):
                ps = ppool.tile([128, L], f32, tag="ps")
                for hc in range(HC):
                    nc.tensor.matmul(
                        ps[:], wq_s[:, hc, oc * 128:(oc + 1) * 128], xT_s[:, hc, :],
                        start=(hc == 0), stop=(hc == HC - 1))
                nc.scalar.activation(qT_s[:, oc, :], ps[:], AF.Identity,
                                     bias=bqs_s[:, oc:oc + 1], scale=inv_sqrt_h)

            # ---- scores + softmax + transpose(probs) per l-half ----
            probsT_s = apool.tile([128, 2, 128], bf16)  # [m, lb, l]
            for lb in range(2):
                pss = ppool.tile([128, Lg], f32, tag="ps")
                for oc in range(OC):
                    nc.tensor.matmul(
                        pss[:], qT_s[:, oc, lb * 128:(lb + 1) * 128], kT_s[:, oc, :],
                        start=(oc == 0), stop=(oc == OC - 1))
                nmax = spool.tile([128, 1], f32, tag="nmax")
                nc.vector.tensor_reduce(nmax[:], pss[:], mybir.AxisListType.X,
                                        OP.max, negate=True)
                e_s = spool.tile([128, Lg], bf16, tag="es")
                ssum = spool.tile([128, 1], f32, tag="ssum")
                nc.scalar.activation(e_s[:], pss[:], AF.Exp,
                                     bias=nmax[:], accum_out=ssum[:])
                rcp = spool.tile([128, 1], f32, tag="rcp")
                nc.vector.reciprocal(rcp[:], ssum[:])
                pr_s = spool.tile([128, Lg], bf16, tag="prs")
                nc.vector.tensor_scalar_mul(pr_s[:], e_s[:], rcp[:])
                pst = ppool.tile([128, 128], bf16, tag="ps")
                nc.tensor.transpose(pst[:], pr_s[:], id_s[:])
                nc.scalar.activation(probsT_s[:, lb, :], pst[:], AF.Copy)

            # ---- v[m,o] (no bias; bv folded into gctx copy) ----
            v_s = apool.tile([128, H], bf16)
            psv = vpool.tile([128, H], f32, tag="psv")
            for third in range(3):
                sl = slice(third * 256, (third + 1) * 256)
                for hc in range(HC):
                    nc.tensor.matmul(psv[:, sl], gT_s[:, hc, :], wv_s[:, hc, sl],
                                     start=(hc == 0), stop=(hc == HC - 1))
            nc.scalar.activation(v_s[:], psv[:], AF.Copy, scale=inv_w8)

            # ---- gctx^T[o,l] (bv folded in via bias) ----
            gcT_s = apool.tile([128, OC, L], bf16)
            for oc in range(OC):
                psg = ppool.tile([128, L], f32, tag="ps")
                for lb in range(2):
                    nc.tensor.matmul(
                        psg[:, lb * 128:(lb + 1) * 128],
                        v_s[:, oc * 128:(oc + 1) * 128], probsT_s[:, lb, :],
                        start=True, stop=True)
                nc.scalar.activation(gcT_s[:, oc, :], psg[:], AF.Identity,
                                     bias=bv_s[:, oc:oc + 1])

            # ---- gate1 + h1^T ----
            h1_s = apool.tile([128, HC, L], bf16)
            for oc in range(OC):
                ps1 = ppool.tile([128, L], f32, tag="ps")
                for hc in range(HC):
                    nc.tensor.matmul(
                        ps1[:], w1x_s[:, hc, oc * 128:(oc + 1) * 128], xT_s[:, hc, :],
                        start=(hc == 0), stop=False)
                for hc in range(HC):
                    nc.tensor.matmul(
                        ps1[:], w1y_s[:, hc, oc * 128:(oc + 1) * 128], gcT_s[:, hc, :],
                        start=False, stop=(hc == HC - 1))
                g1_s = spool.tile([128, L], bf16, tag="g1")
                nc.scalar.activation(g1_s[:], ps1[:], AF.Sigmoid,
                                     bias=bg1_s[:, oc:oc + 1], scale=inv_w8)
                d1 = spool.tile([128, L], bf16, tag="d1")
                nc.vector.tensor_sub(d1[:], xT_s[:, oc, :], gcT_s[:, oc, :])
                m1 = spool.tile([128, L], bf16, tag="m1")
                nc.vector.tensor_mul(m1[:], d1[:], g1_s[:])
                nc.vector.tensor_add(h1_s[:, oc, :], m1[:], gcT_s[:, oc, :])

            # ---- A[oc] and C[oc] for all oc (PE is free; A_sb/cb to SBUF) ----
            A_sb = apool.tile([128, OC, L], bf16)
            cb_s = apool.tile([128, OC, K], f32)
            for oc in range(OC):
                atv = atp.tile([128, L], f32, tag="at")
                for hc in range(HC):
                    nc.tensor.matmul(
                        atv[:], w2x(oc)[:, hc, :], h1_s[:, hc, :],
                        start=(hc == 0), stop=(hc == HC - 1))
                nc.scalar.activation(A_sb[:, oc, :], atv[:], AF.Copy, scale=inv_w8)
                psc = ppool.tile([128, K], f32, tag="ps")
                for hc in range(HC):
                    nc.tensor.matmul(
                        psc[:], w2y(oc)[:, hc, :], sT_s[:, hc, :],
                        start=(hc == 0), stop=(hc == HC - 1))
                nc.scalar.activation(cb_s[:, oc, :], psc[:], AF.Identity,
                                     bias=bg2_s[:, oc:oc + 1], scale=inv_w8)

            # ---- combine per oc ----
            NLC = L // LR
            qrot = [nc.sync, nc.gpsimd]
            for oc in range(OC):
                # sigmoids: k < KF fused on ACT; k >= KF via DVE arg + one ACT
                sig = opool.tile([128, K, L], bf16, tag="sig")
                for k in range(KF):
                    nc.scalar.activation(sig[:, k, :], A_sb[:, oc, :], AF.Sigmoid,
                                         bias=cb_s[:, oc, k:k + 1])
                if KS:
                    crep = spool.tile([128, KS, LR], bf16, tag="crep")
                    nc.vector.tensor_copy(
                        crep[:], cb_s[:, oc, KF:].unsqueeze(2)
                        .broadcast_to([128, KS, LR]))
                    arg = dpool.tile([128, KS, L], bf16, tag="arg")
                    nc.vector.tensor_add(
                        arg[:].rearrange("p k (a b) -> p k a b", a=NLC),
                        A_sb[:, oc, :].rearrange("p (a b) -> p a b", a=NLC)
                        .unsqueeze(1).broadcast_to([128, KS, NLC, LR]),
                        crep[:].unsqueeze(2).broadcast_to([128, KS, NLC, LR]))
                    nc.scalar.activation(
                        sig[:, KF:, :].rearrange("p k l -> p (k l)"),
                        arg[:].rearrange("p k l -> p (k l)"), AF.Sigmoid)

                # m_k = (h1 - s_k) * sig_k : fused per-k STT on DVE
                ob = opool.tile([128, K, L], bf16, tag="ob")
                for k in range(K):
                    nc.vector.scalar_tensor_tensor(
                        ob[:, k, :], h1_s[:, oc, :], sT_s[:, oc, k:k + 1],
                        sig[:, k, :], OP.subtract, OP.mult)

                # out = m + s (batched per k-half on DVE; last GA k's on GPS)
                KH = K // 2
                for kh in range(2):
                    k0, k1 = kh * KH, (kh + 1) * KH
                    kdv = k1 - (GA if kh == 1 else 0)
                    nc.vector.tensor_add(
                        ob[:, k0:kdv, :].rearrange("p k (a b) -> p k a b", a=NLC),
                        ob[:, k0:kdv, :].rearrange("p k (a b) -> p k a b", a=NLC),
                        srep_s[:, oc, k0:kdv].unsqueeze(2)
                        .broadcast_to([128, kdv - k0, NLC, LR]))
                    for k in range(kdv, k1):
                        nc.gpsimd.tensor_scalar_add(
                            ob[:, k, :], ob[:, k, :], sF_s[:, oc, k:k + 1])
                    if oc == OC - 1:
                        KQ = KH // 2
                        for kq in range(2):
                            qs = slice(k0 + kq * KQ, k0 + (kq + 1) * KQ)
                            qrot[(2 * kh + kq) % 2].dma_start(
                                out_d[oc][:, qs], ob[:, qs, :])
                    else:
                        qrot[(2 * oc + kh) % 2].dma_start(
                            out_d[oc][:, k0:k1], ob[:, k0:k1, :])

    nc.compile()
    return nc


def _prep_in_maps(x, s, g, Wq, bq, Wkv, bkv, Wg1, bg1, Wg2, bg2):
    def swz(a):  # [X, H-contract] -> [128, HC, X]: SBUF layout, h on partitions
        aT = np.asarray(a).T  # [H, X]
        return np.ascontiguousarray(
            aT.reshape(HC, 128, -1).transpose(1, 0, 2)).astype(BF16)

    def swz8(a):  # fp8 weight swizzle with x32 prescale
        aT = np.asarray(a).T * W8SCALE
        aT = np.clip(aT, -15.0, 15.0)
        return np.ascontiguousarray(
            aT.reshape(HC, 128, -1).transpose(1, 0, 2)).astype(F8E3)

    def rsh(v):  # (H,) -> [128, H//128] partition-major chunks
        return np.ascontiguousarray(v.reshape(OC, 128).T).astype(np.float32)

    Wk, Wv = Wkv[:H], Wkv[H:]
    W1x, W1y = Wg1[:, :H], Wg1[:, H:]
    W2x, W2y = Wg2[:, :H], Wg2[:, H:]
    w2x8, w2y8 = swz8(W2x), swz8(W2y)  # [128, HC, 768]
    shared = {
        "wq": swz8(Wq), "wk": swz8(Wk), "wv": swz8(Wv),
        "w1x": swz8(W1x), "w1y": swz8(W1y),
    }
    for i in range(3):
        osl = slice(i * 256, (i + 1) * 256)
        shared[f"w2xy{i}"] = np.ascontiguousarray(
            np.stack([w2x8[:, :, osl], w2y8[:, :, osl]], axis=1))
    bstack = np.concatenate(
        [rsh(bq / np.sqrt(H)), rsh(bkv[:H]), rsh(bkv[H:]), rsh(bg1), rsh(bg2)],
        axis=1)
    ident = np.eye(128, dtype=np.float32).astype(BF16)
    in_maps = []
    for b in range(B):
        m = dict(shared)
        # byte-packed small tensors: bstack | xT | gT | sT | ident
        m["acts"] = np.concatenate(
            [bstack.view(np.uint8).reshape(128, -1),
             swz(x[b]).view(np.uint8).reshape(128, -1),
             swz(g[b]).view(np.uint8).reshape(128, -1),
             swz(s[b]).view(np.uint8).reshape(128, -1),
             ident.view(np.uint8).reshape(128, -1)], axis=1)
        # [128, OC, K, LR]: s[k, oc*128+p] replicated along a dummy-l axis
        sr = np.asarray(s[b]).T.reshape(OC, 128, K).transpose(1, 0, 2)
        m["srep"] = np.ascontiguousarray(
            np.broadcast_to(sr[..., None], (128, OC, K, LR))).astype(BF16)
        in_maps.append(m)
    return in_maps


def kernel(**inputs):
    global last_exec_time_ns, last_profile
    from concourse.bass_utils import run_bass_kernel_spmd

    if "nc" not in _CACHE:
        _CACHE["nc"] = _build()
    nc = _CACHE["nc"]

    inputs = {k: np.asarray(v, dtype=np.float32) if np.asarray(v).dtype != np.int32
              else np.asarray(v) for k, v in inputs.items()}
    in_maps = _prep_in_maps(**inputs)

    trace = bool(int(os.environ.get("BASS_KERNEL_TRACE", "0")))
    repeat = int(os.environ.get("BASS_KERNEL_REPEAT", "1"))
    times = []
    for _ in range(repeat):
        res = run_bass_kernel_spmd(nc, in_maps, core_ids=list(range(B)), trace=trace)
        if res.exec_time_ns is not None:
            times.append(res.exec_time_ns)
    if times:
        print(f"exec times: {times}")
        last_exec_time_ns = min(times)
    last_profile = res.profile_json

    out = np.empty((B, L, K, H), dtype=np.float32)
    for b in range(B):
        # per-core result is [OC, 128, K, L] -> [L, K, H]
        r = res.results[b]["out"].astype(np.float32)
        out[b] = np.transpose(r, (3, 2, 0, 1)).reshape(L, K, H)
    return out


# revision 8
# speedup vs baseline: 1.2583x; 1.2583x over previous
"""Trainium2 Bass kernel for nn_AdaptiveFusion.

Math (per batch b):
  q  = x @ Wq.T + bq                         (L,H)
  kv = g @ Wkv.T + bkv ; k,v = split         (Lg,H) each
  p  = softmax(q @ k.T / sqrt(H))            (L,Lg)
  gc = p @ v                                 (L,H)
  g1 = sigmoid(x @ W1x.T + gc @ W1y.T + bg1) (L,H)   [k-independent]
  h1 = gc + g1*(x - gc)                      (L,H)
  A  = h1 @ W2x.T                            (L,H)
  C  = s @ W2y.T + bg2                       (K,H)
  out[l,k,o] = s[k,o] + sigmoid(A[l,o]+C[k,o]) * (h1[l,o]-s[k,o])

Sharding: data-parallel over B (8 batches -> 8 cores), weights replicated,
no collectives.

v3 structure:
  - all weights fp8-e3m4 (x32 prescale, descaled free via ACT scale)
  - PE warm-up dummy matmuls from t=0 so the real stream runs at full clock
  - need-ordered single-queue input DMA; w2x/w2y interleaved per-oc-pair
  - full-L chain (minimal LDWEIGHTS count), then A/C for all oc hoisted
  - combine per oc:
      k <  KF : sig_k = Sigmoid(A + C_k) per-k on ACT (bias trick)
      k >= KF : arg = A + C_rep via one DVE TT, one batched ACT sigmoid
      m_k     = (h1 - s_k) * sig_k  -- per-k DVE scalar_tensor_tensor
                (fuses the old sub+mul passes; scalar s_k comes from sT)
      out     = m + s_rep -- batched DVE TT per k-half; last GA k's of each
                oc instead take a GPSIMD per-k tensor_scalar add
Output DMA in [OC, 128, K, L] layout (the SBUF tile layout -> fully linear
descriptors); the host permutes back.
"""

import os
import sys

import numpy as np

if "/opt/trn_rl_repo" not in sys.path:
    sys.path.insert(0, "/opt/trn_rl_repo")

import ml_dtypes

BF16 = ml_dtypes.bfloat16
F8E3 = ml_dtypes.float8_e3m4

B, L, K, Lg, H = 8, 256, 32, 128, 768
HC = H // 128  # h-chunks
OC = H // 128  # o-chunks
LR = 8         # dummy-l width of the host-replicated s
KF = 19        # k's handled by fused per-k ACT sigmoid; rest via DVE arg
KS = K - KF
GA = 4         # trailing k's per oc whose final add runs on GPSIMD
W8SCALE = 32.0
DUMN = 24      # PE warm-up dummy matmuls

_CACHE = {}

last_exec_time_ns = None
last_profile = None


def _build():
    import concourse.bacc as bacc
    import concourse.bass as bass
    import concourse.mybir as mybir
    import concourse.tile as tile

    f32 = mybir.dt.float32
    bf16 = mybir.dt.bfloat16
    fp8 = mybir.dt.float8e3
    AF = mybir.ActivationFunctionType
    OP = mybir.AluOpType

    nc = bacc.Bacc(None, target_bir_lowering=False, debug=False)

    u8 = mybir.dt.uint8
    # acts layout per partition: bstack f32 (120B) | xT (3072B) | gT (1536B)
    #                            | sT (384B) | ident (256B)
    ABYTES = 120 + 3072 + 1536 + 384 + 256
    acts = nc.declare_dram_parameter("acts", [128, ABYTES], u8, isOutput=False)
    srep = nc.declare_dram_parameter("srep", [128, OC, K, LR], bf16, isOutput=False)
    wq_d = nc.declare_dram_parameter("wq", [128, HC, H], fp8, isOutput=False)
    wk_d = nc.declare_dram_parameter("wk", [128, HC, H], fp8, isOutput=False)
    wv_d = nc.declare_dram_parameter("wv", [128, HC, H], fp8, isOutput=False)
    w1x_d = nc.declare_dram_parameter("w1x", [128, HC, H], fp8, isOutput=False)
    w1y_d = nc.declare_dram_parameter("w1y", [128, HC, H], fp8, isOutput=False)
    # w2x/w2y interleaved per oc-pair so the first combine unblocks early
    w2xy_d = [nc.declare_dram_parameter(f"w2xy{i}", [128, 2, HC, 256], fp8,
                                        isOutput=False) for i in range(3)]
    out_d = nc.declare_dram_parameter("out", [OC, 128, K, L], bf16, isOutput=True)

    inv_w8 = 1.0 / W8SCALE
    inv_sqrt_h = 1.0 / float(np.sqrt(H)) * inv_w8

    with tile.TileContext(nc) as tc:
        with (
            tc.tile_pool(name="wpool", bufs=1) as wpool,
            tc.tile_pool(name="apool", bufs=1) as apool,
            tc.tile_pool(name="ppool", bufs=3, space=bass.MemorySpace.PSUM) as ppool,
            tc.tile_pool(name="vpool", bufs=1, space=bass.MemorySpace.PSUM) as vpool,
            tc.tile_pool(name="atp", bufs=2, space=bass.MemorySpace.PSUM) as atp,
            tc.tile_pool(name="wmp", bufs=1, space=bass.MemorySpace.PSUM) as wmp,
            tc.tile_pool(name="spool", bufs=3) as spool,
            tc.tile_pool(name="opool", bufs=2) as opool,
            tc.tile_pool(name="dpool", bufs=2) as dpool,
            tc.tile_pool(name="hpool", bufs=1) as hpool,
        ):
            # ---- PE warm-up: dummy matmuls with no external deps ----
            dum = spool.tile([128, 256], bf16, tag="dum")
            nc.vector.memset(dum[:], 0)
            for i in range(DUMN):
                psd = wmp.tile([128, 256], f32, tag="psd")
                nc.tensor.matmul(psd[:], dum[:, 0:128], dum[:], start=True,
                                 stop=True)

            # ---- input DMAs: one queue (sync), strict need order ----
            acts_s = apool.tile([128, ABYTES], u8)
            nc.sync.dma_start(acts_s[:], acts[:])
            bst_s = acts_s[:, 0:120].bitcast(f32)
            xT_s = acts_s[:, 120:3192].bitcast(bf16).rearrange(
                "p (c l) -> p c l", c=HC)
            gT_s = acts_s[:, 3192:4728].bitcast(bf16).rearrange(
                "p (c l) -> p c l", c=HC)
            sT_s = acts_s[:, 4728:5112].bitcast(bf16).rearrange(
                "p (c l) -> p c l", c=HC)
            id_s = acts_s[:, 5112:5368].bitcast(bf16)

            def wload(name, src):
                t = wpool.tile([128, HC, H], fp8, tag=name)
                nc.sync.dma_start(t[:], src[:])
                return t

            wk_s = wload("wk", wk_d)
            wq_s = wload("wq", wq_d)
            wv_s = wload("wv", wv_d)
            w1y_s = wload("w1y", w1y_d)
            w1x_s = wload("w1x", w1x_d)
            srep_s = apool.tile([128, OC, K, LR], bf16)
            nc.sync.dma_start(srep_s[:], srep[:])
            w2xy_s = []
            for i in range(3):
                t = wpool.tile([128, 2, HC, 256], fp8, tag=f"w2xy{i}")
                nc.sync.dma_start(t[:], w2xy_d[i][:])
                w2xy_s.append(t)

            def w2x(oc):  # [128(h), HC, 128] fp8 slice for output chunk oc
                return w2xy_s[oc // 2][:, 0, :, (oc % 2) * 128:(oc % 2) * 128 + 128]

            def w2y(oc):
                return w2xy_s[oc // 2][:, 1, :, (oc % 2) * 128:(oc % 2) * 128 + 128]

            bqs_s, bk_s, bv_s = bst_s[:, 0:OC], bst_s[:, OC:2 * OC], bst_s[:, 2 * OC:3 * OC]
            bg1_s, bg2_s = bst_s[:, 3 * OC:4 * OC], bst_s[:, 4 * OC:5 * OC]

            # warm only the Exp table; the Sigmoid set loads once later
            scratch = spool.tile([128, 2], f32, tag="warm")
            nc.scalar.activation(scratch[:, 0:1], bst_s[:, 0:1], AF.Exp)

            # f32 copy of s^T for GPSIMD tensor_scalar adds
            sF_s = apool.tile([128, OC, K], f32)
            nc.vector.tensor_copy(sF_s[:], sT_s[:])

            # ---- k^T[o,m] ----
            kT_s = apool.tile([128, OC, Lg], bf16)
            for oc in range(OC):
                ps = ppool.tile([128, Lg], f32, tag="ps")
                for hc in range(HC):
                    nc.tensor.matmul(
                        ps[:], wk_s[:, hc, oc * 128:(oc + 1) * 128], gT_s[:, hc, :],
                        start=(hc == 0), stop=(hc == HC - 1))
                nc.scalar.activation(kT_s[:, oc, :], ps[:], AF.Identity,
                                     bias=bk_s[:, oc:oc + 1], scale=inv_w8)

            # ---- q^T[o,l] ----
            qT_s = apool.tile([128, OC, L], bf16)
            for oc in range(OC):
                ps = ppool.tile([128, L], f32, tag="ps")
                for hc in range(HC):
                    nc.tensor.matmul(
                        ps[:], wq_s[:, hc, oc * 128:(oc + 1) * 128], xT_s[:, hc, :],
                        start=(hc == 0), stop=(hc == HC - 1))
                nc.scalar.activation(qT_s[:, oc, :], ps[:], AF.Identity,
                                     bias=bqs_s[:, oc:oc + 1], scale=inv_sqrt_h)

            # ---- scores + softmax + transpose(probs) per l-half ----
            probsT_s = apool.tile([128, 2, 128], bf16)  # [m, lb, l]
            for lb in range(2):
                pss = ppool.tile([128, Lg], f32, tag="ps")
                for oc in range(OC):
                    nc.tensor.matmul(
                        pss[:], qT_s[:, oc, lb * 128:(lb + 1) * 128], kT_s[:, oc, :],
                        start=(oc == 0), stop=(oc == OC - 1))
                nmax = spool.tile([128, 1], f32, tag="nmax")
                nc.vector.tensor_reduce(nmax[:], pss[:], mybir.AxisListType.X,
                                        OP.max, negate=True)
                e_s = spool.tile([128, Lg], bf16, tag="es")
                ssum = spool.tile([128, 1], f32, tag="ssum")
                nc.scalar.activation(e_s[:], pss[:], AF.Exp,
                                     bias=nmax[:], accum_out=ssum[:])
                rcp = spool.tile([128, 1], f32, tag="rcp")
                nc.vector.reciprocal(rcp[:], ssum[:])
                pr_s = spool.tile([128, Lg], bf16, tag="prs")
                nc.vector.tensor_scalar_mul(pr_s[:], e_s[:], rcp[:])
                pst = ppool.tile([128, 128], bf16, tag="ps")
                nc.tensor.transpose(pst[:], pr_s[:], id_s[:])
                nc.scalar.activation(probsT_s[:, lb, :], pst[:], AF.Copy)

            # ---- v[m,o] (no bias; bv folded into gctx copy) ----
            v_s = apool.tile([128, H], bf16)
            psv = vpool.tile([128, H], f32, tag="psv")
            for third in range(3):
                sl = slice(third * 256, (third + 1) * 256)
                for hc in range(HC):
                    nc.tensor.matmul(psv[:, sl], gT_s[:, hc, :], wv_s[:, hc, sl],
                                     start=(hc == 0), stop=(hc == HC - 1))
            nc.scalar.activation(v_s[:], psv[:], AF.Copy, scale=inv_w8)

            # ---- gctx^T[o,l] (bv folded in via bias) ----
            gcT_s = apool.tile([128, OC, L], bf16)
            for oc in range(OC):
                psg = ppool.tile([128, L], f32, tag="ps")
                for lb in range(2):
                    nc.tensor.matmul(
                        psg[:, lb * 128:(lb + 1) * 128],
                        v_s[:, oc * 128:(oc + 1) * 128], probsT_s[:, lb, :],
                        start=True, stop=True)
                nc.scalar.activation(gcT_s[:, oc, :], psg[:], AF.Identity,
                                     bias=bv_s[:, oc:oc + 1])

            # ---- gate1 + h1^T ----
            h1_s = apool.tile([128, HC, L], bf16)
            for oc in range(OC):
                ps1 = ppool.tile([128, L], f32, tag="ps")
                for hc in range(HC):
                    nc.tensor.matmul(
                        ps1[:], w1x_s[:, hc, oc * 128:(oc + 1) * 128], xT_s[:, hc, :],
                        start=(hc == 0), stop=False)
                for hc in range(HC):
                    nc.tensor.matmul(
                        ps1[:], w1y_s[:, hc, oc * 128:(oc + 1) * 128], gcT_s[:, hc, :],
                        start=False, stop=(hc == HC - 1))
                g1_s = spool.tile([128, L], bf16, tag="g1")
                nc.scalar.activation(g1_s[:], ps1[:], AF.Sigmoid,
                                     bias=bg1_s[:, oc:oc + 1], scale=inv_w8)
                d1 = spool.tile([128, L], bf16, tag="d1")
                nc.vector.tensor_sub(d1[:], xT_s[:, oc, :], gcT_s[:, oc, :])
                m1 = spool.tile([128, L], bf16, tag="m1")
                nc.vector.tensor_mul(m1[:], d1[:], g1_s[:])
                nc.vector.tensor_add(h1_s[:, oc, :], m1[:], gcT_s[:, oc, :])

            # ---- A[oc] and C[oc] for all oc (PE is free; A_sb/cb to SBUF) ----
            A_sb = apool.tile([128, OC, L], bf16)
            cb_s = apool.tile([128, OC, K], f32)
            for oc in range(OC):
                atv = atp.tile([128, L], f32, tag="at")
                for hc in range(HC):
                    nc.tensor.matmul(
                        atv[:], w2x(oc)[:, hc, :], h1_s[:, hc, :],
                        start=(hc == 0), stop=(hc == HC - 1))
                nc.scalar.activation(A_sb[:, oc, :], atv[:], AF.Copy, scale=inv_w8)
                psc = ppool.tile([128, K], f32, tag="ps")
                for hc in range(HC):
                    nc.tensor.matmul(
                        psc[:], w2y(oc)[:, hc, :], sT_s[:, hc, :],
                        start=(hc == 0), stop=(hc == HC - 1))
                nc.scalar.activation(cb_s[:, oc, :], psc[:], AF.Identity,
                                     bias=bg2_s[:, oc:oc + 1], scale=inv_w8)

            # ---- combine per oc ----
            NLC = L // LR

            # hoist the last oc's d=h1-s sub so the tail has no big
            # DVE op left after the final sigmoid
            dbuf_hoist = {}
            for oc in (5,):
                db = hpool.tile([128, K, L], bf16, tag=f"dbh{oc}")
                nc.vector.tensor_sub(
                    db[:].rearrange("p k (a b) -> p k a b", a=NLC),
                    h1_s[:, oc, :].rearrange("p (a b) -> p a b", a=NLC)
                    .unsqueeze(1).broadcast_to([128, K, NLC, LR]),
                    srep_s[:, oc].unsqueeze(2).broadcast_to([128, K, NLC, LR]))
                dbuf_hoist[oc] = db

            for oc in range(OC):
                if oc in dbuf_hoist:
                    dbuf = dbuf_hoist[oc]
                else:
                    dbuf = dpool.tile([128, K, L], bf16, tag="dbuf")
                    nc.vector.tensor_sub(
                        dbuf[:].rearrange("p k (a b) -> p k a b", a=NLC),
                        h1_s[:, oc, :].rearrange("p (a b) -> p a b", a=NLC)
                        .unsqueeze(1).broadcast_to([128, K, NLC, LR]),
                        srep_s[:, oc].unsqueeze(2).broadcast_to([128, K, NLC, LR]))

                # sigmoids: k < KF fused on ACT; k >= KF via DVE arg + one ACT
                sig = opool.tile([128, K, L], bf16, tag="sig")
                for k in range(KF):
                    nc.scalar.activation(sig[:, k, :], A_sb[:, oc, :], AF.Sigmoid,
                                         bias=cb_s[:, oc, k:k + 1])
                if KS:
                    crep = spool.tile([128, KS, LR], bf16, tag="crep")
                    nc.vector.tensor_copy(
                        crep[:], cb_s[:, oc, KF:].unsqueeze(2)
                        .broadcast_to([128, KS, LR]))
                    arg = dpool.tile([128, KS, L], bf16, tag="arg")
                    nc.vector.tensor_add(
                        arg[:].rearrange("p k (a b) -> p k a b", a=NLC),
                        A_sb[:, oc, :].rearrange("p (a b) -> p a b", a=NLC)
                        .unsqueeze(1).broadcast_to([128, KS, NLC, LR]),
                        crep[:].unsqueeze(2).broadcast_to([128, KS, NLC, LR]))
                    nc.scalar.activation(
                        sig[:, KF:, :].rearrange("p k l -> p (k l)"),
                        arg[:].rearrange("p k l -> p (k l)"), AF.Sigmoid)

                # m = d * sig; out = m + s; DMA per chunk (k-quarters for the
                # last oc so its pipeline drains fast)
                ob = opool.tile([128, K, L], bf16, tag="ob")
                nch = 4 if oc == OC - 1 else 2
                KC = K // nch
                for kc in range(nch):
                    k0, k1 = kc * KC, (kc + 1) * KC
                    kdv = k1 - (GA if k1 == K else 0)
                    nc.vector.tensor_mul(
                        ob[:, k0:k1, :].rearrange("p k l -> p (k l)"),
                        dbuf[:, k0:k1, :].rearrange("p k l -> p (k l)"),
                        sig[:, k0:k1, :].rearrange("p k l -> p (k l)"))
                    nc.vector.tensor_add(
                        ob[:, k0:kdv, :].rearrange("p k (a b) -> p k a b", a=NLC),
                        ob[:, k0:kdv, :].rearrange("p k (a b) -> p k a b", a=NLC),
                        srep_s[:, oc, k0:kdv].unsqueeze(2)
                        .broadcast_to([128, kdv - k0, NLC, LR]))
                    for k in range(kdv, k1):
                        nc.gpsimd.tensor_scalar_add(
                            ob[:, k, :], ob[:, k, :], sF_s[:, oc, k:k + 1])
                    nc.sync.dma_start(out_d[oc][:, k0:k1], ob[:, k0:k1, :])

    nc.compile()
    return nc


def _prep_in_maps(x, s, g, Wq, bq, Wkv, bkv, Wg1, bg1, Wg2, bg2):
    def swz(a):  # [X, H-contract] -> [128, HC, X]: SBUF layout, h on partitions
        aT = np.asarray(a).T  # [H, X]
        return np.ascontiguousarray(
            aT.reshape(HC, 128, -1).transpose(1, 0, 2)).astype(BF16)

    def swz8(a):  # fp8 weight swizzle with x32 prescale
        aT = np.asarray(a).T * W8SCALE
        aT = np.clip(aT, -15.0, 15.0)
        return np.ascontiguousarray(
            aT.reshape(HC, 128, -1).transpose(1, 0, 2)).astype(F8E3)

    def rsh(v):  # (H,) -> [128, H//128] partition-major chunks
        return np.ascontiguousarray(v.reshape(OC, 128).T).astype(np.float32)

    Wk, Wv = Wkv[:H], Wkv[H:]
    W1x, W1y = Wg1[:, :H], Wg1[:, H:]
    W2x, W2y = Wg2[:, :H], Wg2[:, H:]
    w2x8, w2y8 = swz8(W2x), swz8(W2y)  # [128, HC, 768]
    shared = {
        "wq": swz8(Wq), "wk": swz8(Wk), "wv": swz8(Wv),
        "w1x": swz8(W1x), "w1y": swz8(W1y),
    }
    for i in range(3):
        osl = slice(i * 256, (i + 1) * 256)
        shared[f"w2xy{i}"] = np.ascontiguousarray(
            np.stack([w2x8[:, :, osl], w2y8[:, :, osl]], axis=1))
    bstack = np.concatenate(
        [rsh(bq / np.sqrt(H)), rsh(bkv[:H]), rsh(bkv[H:]), rsh(bg1), rsh(bg2)],
        axis=1)
    ident = np.eye(128, dtype=np.float32).astype(BF16)
    in_maps = []
    for b in range(B):
        m = dict(shared)
        # byte-packed small tensors: bstack | xT | gT | sT | ident
        m["acts"] = np.concatenate(
            [bstack.view(np.uint8).reshape(128, -1),
             swz(x[b]).view(np.uint8).reshape(128, -1),
             swz(g[b]).view(np.uint8).reshape(128, -1),
             swz(s[b]).view(np.uint8).reshape(128, -1),
             ident.view(np.uint8).reshape(128, -1)], axis=1)
        # [128, OC, K, LR]: s[k, oc*128+p] replicated along a dummy-l axis
        sr = np.asarray(s[b]).T.reshape(OC, 128, K).transpose(1, 0, 2)
        m["srep"] = np.ascontiguousarray(
            np.broadcast_to(sr[..., None], (128, OC, K, LR))).astype(BF16)
        in_maps.append(m)
    return in_maps


def kernel(**inputs):
    global last_exec_time_ns, last_profile
    from concourse.bass_utils import run_bass_kernel_spmd

    if "nc" not in _CACHE:
        _CACHE["nc"] = _build()
    nc = _CACHE["nc"]

    inputs = {k: np.asarray(v, dtype=np.float32) if np.asarray(v).dtype != np.int32
              else np.asarray(v) for k, v in inputs.items()}
    in_maps = _prep_in_maps(**inputs)

    trace = bool(int(os.environ.get("BASS_KERNEL_TRACE", "0")))
    repeat = int(os.environ.get("BASS_KERNEL_REPEAT", "1"))
    times = []
    for _ in range(repeat):
        res = run_bass_kernel_spmd(nc, in_maps, core_ids=list(range(B)), trace=trace)
        if res.exec_time_ns is not None:
            times.append(res.exec_time_ns)
    if times:
        print(f"exec times: {times}")
        last_exec_time_ns = min(times)
    last_profile = res.profile_json

    out = np.empty((B, L, K, H), dtype=np.float32)
    for b in range(B):
        # per-core result is [OC, 128, K, L] -> [L, K, H]
        r = res.results[b]["out"].astype(np.float32)
        out[b] = np.transpose(r, (3, 2, 0, 1)).reshape(L, K, H)
    return out


# revision 9
# speedup vs baseline: 1.8915x; 1.5033x over previous
"""Trainium2 Bass kernel for nn_AdaptiveFusion.

Math (per batch b):
  q  = x @ Wq.T + bq                         (L,H)
  kv = g @ Wkv.T + bkv ; k,v = split         (Lg,H) each
  p  = softmax(q @ k.T / sqrt(H))            (L,Lg)
  gc = p @ v                                 (L,H)
  g1 = sigmoid(x @ W1x.T + gc @ W1y.T + bg1) (L,H)   [k-independent]
  h1 = gc + g1*(x - gc)                      (L,H)
  A  = h1 @ W2x.T                            (L,H)
  C  = s @ W2y.T + bg2                       (K,H)
  out[l,k,o] = s[k,o] + sigmoid(A[l,o]+C[k,o]) * (h1[l,o]-s[k,o])

Sharding: data-parallel over B (8 batches -> 8 cores), weights replicated,
no collectives.

v3 structure:
  - all weights fp8-e3m4 (x32 prescale, descaled free via ACT scale)
  - PE warm-up dummy matmuls from t=0 so the real stream runs at full clock
  - need-ordered single-queue input DMA; w2x/w2y interleaved per-oc-pair
  - full-L chain (minimal LDWEIGHTS count), then A/C for all oc hoisted
  - combine per oc:
      k <  KF : sig_k = Sigmoid(A + C_k) per-k on ACT (bias trick)
      k >= KF : arg = A + C_rep via one DVE TT, one batched ACT sigmoid
      m_k     = (h1 - s_k) * sig_k  -- per-k DVE scalar_tensor_tensor
                (fuses the old sub+mul passes; scalar s_k comes from sT)
      out     = m + s_rep -- batched DVE TT per k-half; last GA k's of each
                oc instead take a GPSIMD per-k tensor_scalar add
Output DMA in [OC, 128, K, L] layout (the SBUF tile layout -> fully linear
descriptors); the host permutes back.
"""

import os
import sys

import numpy as np

if "/opt/trn_rl_repo" not in sys.path:
    sys.path.insert(0, "/opt/trn_rl_repo")

import ml_dtypes

BF16 = ml_dtypes.bfloat16
F8E3 = ml_dtypes.float8_e3m4

B, L, K, Lg, H = 8, 256, 32, 128, 768
HC = H // 128  # h-chunks
OC = H // 128  # o-chunks
LR = 8         # dummy-l width of the host-replicated s
KF = 19        # k's handled by fused per-k ACT sigmoid; rest via DVE arg
KS = K - KF
GA = 0         # trailing k's per oc whose final add runs on GPSIMD
W8SCALE = 32.0
DUMN = 24      # PE warm-up dummy matmuls

_CACHE = {}

last_exec_time_ns = None
last_profile = None


def _build():
    import concourse.bacc as bacc
    import concourse.bass as bass
    import concourse.mybir as mybir
    import concourse.tile as tile

    f32 = mybir.dt.float32
    bf16 = mybir.dt.bfloat16
    fp8 = mybir.dt.float8e3
    AF = mybir.ActivationFunctionType
    OP = mybir.AluOpType

    nc = bacc.Bacc(None, target_bir_lowering=False, debug=False)

    u8 = mybir.dt.uint8
    # acts layout per partition: bstack f32 (120B) | xT (3072B) | gT (1536B)
    #                            | sT (384B) | ident (256B)
    ABYTES = 120 + 3072 + 1536 + 384 + 256
    acts = nc.declare_dram_parameter("acts", [128, ABYTES], u8, isOutput=False)
    srep = nc.declare_dram_parameter("srep", [128, OC, K, LR], bf16, isOutput=False)
    wq_d = nc.declare_dram_parameter("wq", [128, HC, H], fp8, isOutput=False)
    wk_d = nc.declare_dram_parameter("wk", [128, HC, H], fp8, isOutput=False)
    wv_d = nc.declare_dram_parameter("wv", [128, HC, H], fp8, isOutput=False)
    w1x_d = nc.declare_dram_parameter("w1x", [128, HC, H], fp8, isOutput=False)
    w1y_d = nc.declare_dram_parameter("w1y", [128, HC, H], fp8, isOutput=False)
    # w2x/w2y interleaved per oc-pair so the first combine unblocks early
    w2xy_d = [nc.declare_dram_parameter(f"w2xy{i}", [128, 2, HC, 256], fp8,
                                        isOutput=False) for i in range(3)]
    out_d = nc.declare_dram_parameter("out", [OC, 128, K, L], bf16, isOutput=True)

    inv_w8 = 1.0 / W8SCALE
    inv_sqrt_h = 1.0 / float(np.sqrt(H)) * inv_w8

    with tile.TileContext(nc) as tc:
        with (
            tc.tile_pool(name="wpool", bufs=1) as wpool,
            tc.tile_pool(name="apool", bufs=1) as apool,
            tc.tile_pool(name="ppool", bufs=3, space=bass.MemorySpace.PSUM) as ppool,
            tc.tile_pool(name="vpool", bufs=1, space=bass.MemorySpace.PSUM) as vpool,
            tc.tile_pool(name="atp", bufs=2, space=bass.MemorySpace.PSUM) as atp,
            tc.tile_pool(name="wmp", bufs=1, space=bass.MemorySpace.PSUM) as wmp,
            tc.tile_pool(name="spool", bufs=3) as spool,
            tc.tile_pool(name="opool", bufs=2) as opool,
            tc.tile_pool(name="dpool", bufs=2) as dpool,
            tc.tile_pool(name="hpool", bufs=1) as hpool,
        ):
            # ---- PE warm-up: dummy matmuls with no external deps ----
            dum = spool.tile([128, 256], bf16, tag="dum")
            nc.vector.memset(dum[:], 0)
            for i in range(DUMN):
                psd = wmp.tile([128, 256], f32, tag="psd")
                nc.tensor.matmul(psd[:], dum[:, 0:128], dum[:], start=True,
                                 stop=True)

            # ---- input DMAs: one queue (sync), strict need order ----
            acts_s = apool.tile([128, ABYTES], u8)
            nc.sync.dma_start(acts_s[:], acts[:])
            bst_s = acts_s[:, 0:120].bitcast(f32)
            xT_s = acts_s[:, 120:3192].bitcast(bf16).rearrange(
                "p (c l) -> p c l", c=HC)
            gT_s = acts_s[:, 3192:4728].bitcast(bf16).rearrange(
                "p (c l) -> p c l", c=HC)
            sT_s = acts_s[:, 4728:5112].bitcast(bf16).rearrange(
                "p (c l) -> p c l", c=HC)
            id_s = acts_s[:, 5112:5368].bitcast(bf16)

            def wload(name, src):
                t = wpool.tile([128, HC, H], fp8, tag=name)
                nc.sync.dma_start(t[:], src[:])
                return t

            wk_s = wload("wk", wk_d)
            wq_s = wload("wq", wq_d)
            wv_s = wload("wv", wv_d)
            w1y_s = wload("w1y", w1y_d)
            w1x_s = wload("w1x", w1x_d)
            srep_s = apool.tile([128, OC, K, LR], bf16)
            nc.sync.dma_start(srep_s[:], srep[:])
            w2xy_s = []
            for i in range(3):
                t = wpool.tile([128, 2, HC, 256], fp8, tag=f"w2xy{i}")
                nc.sync.dma_start(t[:], w2xy_d[i][:])
                w2xy_s.append(t)

            def w2x(oc):  # [128(h), HC, 128] fp8 slice for output chunk oc
                return w2xy_s[oc // 2][:, 0, :, (oc % 2) * 128:(oc % 2) * 128 + 128]

            def w2y(oc):
                return w2xy_s[oc // 2][:, 1, :, (oc % 2) * 128:(oc % 2) * 128 + 128]

            bqs_s, bk_s, bv_s = bst_s[:, 0:OC], bst_s[:, OC:2 * OC], bst_s[:, 2 * OC:3 * OC]
            bg1_s, bg2_s = bst_s[:, 3 * OC:4 * OC], bst_s[:, 4 * OC:5 * OC]

            # warm only the Exp table; the Sigmoid set loads once later
            scratch = spool.tile([128, 2], f32, tag="warm")
            nc.scalar.activation(scratch[:, 0:1], bst_s[:, 0:1], AF.Exp)

            # f32 copy of s^T for GPSIMD tensor_scalar adds
            sF_s = apool.tile([128, OC, K], f32)
            nc.vector.tensor_copy(sF_s[:], sT_s[:])

            # ---- k^T[o,m] ----
            kT_s = apool.tile([128, OC, Lg], bf16)
            for oc in range(OC):
                ps = ppool.tile([128, Lg], f32, tag="ps")
                for hc in range(HC):
                    nc.tensor.matmul(
                        ps[:], wk_s[:, hc, oc * 128:(oc + 1) * 128], gT_s[:, hc, :],
                        start=(hc == 0), stop=(hc == HC - 1))
                nc.scalar.activation(kT_s[:, oc, :], ps[:], AF.Identity,
                                     bias=bk_s[:, oc:oc + 1], scale=inv_w8)

            # ---- q^T[o,l] ----
            qT_s = apool.tile([128, OC, L], bf16)
            for oc in range(OC):
                ps = ppool.tile([128, L], f32, tag="ps")
                for hc in range(HC):
                    nc.tensor.matmul(
                        ps[:], wq_s[:, hc, oc * 128:(oc + 1) * 128], xT_s[:, hc, :],
                        start=(hc == 0), stop=(hc == HC - 1))
                nc.scalar.activation(qT_s[:, oc, :], ps[:], AF.Identity,
                                     bias=bqs_s[:, oc:oc + 1], scale=inv_sqrt_h)

            # ---- scores + softmax + transpose(probs) per l-half ----
            probsT_s = apool.tile([128, 2, 128], bf16)  # [m, lb, l]
            for lb in range(2):
                pss = ppool.tile([128, Lg], f32, tag="ps")
                for oc in range(OC):
                    nc.tensor.matmul(
                        pss[:], qT_s[:, oc, lb * 128:(lb + 1) * 128], kT_s[:, oc, :],
                        start=(oc == 0), stop=(oc == OC - 1))
                nmax = spool.tile([128, 1], f32, tag="nmax")
                nc.vector.tensor_reduce(nmax[:], pss[:], mybir.AxisListType.X,
                                        OP.max, negate=True)
                e_s = spool.tile([128, Lg], bf16, tag="es")
                ssum = spool.tile([128, 1], f32, tag="ssum")
                nc.scalar.activation(e_s[:], pss[:], AF.Exp,
                                     bias=nmax[:], accum_out=ssum[:])
                rcp = spool.tile([128, 1], f32, tag="rcp")
                nc.vector.reciprocal(rcp[:], ssum[:])
                pr_s = spool.tile([128, Lg], bf16, tag="prs")
                nc.vector.tensor_scalar_mul(pr_s[:], e_s[:], rcp[:])
                pst = ppool.tile([128, 128], bf16, tag="ps")
                nc.tensor.transpose(pst[:], pr_s[:], id_s[:])
                nc.scalar.activation(probsT_s[:, lb, :], pst[:], AF.Copy)

            # ---- v[m,o] (no bias; bv folded into gctx copy) ----
            v_s = apool.tile([128, H], bf16)
            psv = vpool.tile([128, H], f32, tag="psv")
            for third in range(3):
                sl = slice(third * 256, (third + 1) * 256)
                for hc in range(HC):
                    nc.tensor.matmul(psv[:, sl], gT_s[:, hc, :], wv_s[:, hc, sl],
                                     start=(hc == 0), stop=(hc == HC - 1))
            nc.scalar.activation(v_s[:], psv[:], AF.Copy, scale=inv_w8)

            # ---- gctx^T[o,l] (bv folded in via bias) ----
            gcT_s = apool.tile([128, OC, L], bf16)
            for oc in range(OC):
                psg = ppool.tile([128, L], f32, tag="ps")
                for lb in range(2):
                    nc.tensor.matmul(
                        psg[:, lb * 128:(lb + 1) * 128],
                        v_s[:, oc * 128:(oc + 1) * 128], probsT_s[:, lb, :],
                        start=True, stop=True)
                nc.scalar.activation(gcT_s[:, oc, :], psg[:], AF.Identity,
                                     bias=bv_s[:, oc:oc + 1])

            # ---- gate1 + h1^T ----
            h1_s = apool.tile([128, HC, L], bf16)
            for oc in range(OC):
                ps1 = ppool.tile([128, L], f32, tag="ps")
                for hc in range(HC):
                    nc.tensor.matmul(
                        ps1[:], w1x_s[:, hc, oc * 128:(oc + 1) * 128], xT_s[:, hc, :],
                        start=(hc == 0), stop=False)
                for hc in range(HC):
                    nc.tensor.matmul(
                        ps1[:], w1y_s[:, hc, oc * 128:(oc + 1) * 128], gcT_s[:, hc, :],
                        start=False, stop=(hc == HC - 1))
                g1_s = spool.tile([128, L], bf16, tag="g1")
                nc.scalar.activation(g1_s[:], ps1[:], AF.Sigmoid,
                                     bias=bg1_s[:, oc:oc + 1], scale=inv_w8)
                d1 = spool.tile([128, L], bf16, tag="d1")
                nc.vector.tensor_sub(d1[:], xT_s[:, oc, :], gcT_s[:, oc, :])
                m1 = spool.tile([128, L], bf16, tag="m1")
                nc.vector.tensor_mul(m1[:], d1[:], g1_s[:])
                nc.vector.tensor_add(h1_s[:, oc, :], m1[:], gcT_s[:, oc, :])

            # ---- A[oc] and C[oc] for all oc (PE is free; A_sb/cb to SBUF) ----
            A_sb = apool.tile([128, OC, L], bf16)
            cb_s = apool.tile([128, OC, K], f32)
            for oc in range(OC):
                atv = atp.tile([128, L], f32, tag="at")
                for hc in range(HC):
                    nc.tensor.matmul(
                        atv[:], w2x(oc)[:, hc, :], h1_s[:, hc, :],
                        start=(hc == 0), stop=(hc == HC - 1))
                nc.scalar.activation(A_sb[:, oc, :], atv[:], AF.Copy, scale=inv_w8)
                psc = ppool.tile([128, K], f32, tag="ps")
                for hc in range(HC):
                    nc.tensor.matmul(
                        psc[:], w2y(oc)[:, hc, :], sT_s[:, hc, :],
                        start=(hc == 0), stop=(hc == HC - 1))
                nc.scalar.activation(cb_s[:, oc, :], psc[:], AF.Identity,
                                     bias=bg2_s[:, oc:oc + 1], scale=inv_w8)

            # ---- combine per oc ----
            NLC = L // LR

            # hoist the last oc's d=h1-s sub so the tail has no big
            # DVE op left after the final sigmoid
            dbuf_hoist = {}
            for oc in (5,):
                db = hpool.tile([128, K, L], bf16, tag=f"dbh{oc}")
                nc.vector.tensor_sub(
                    db[:].rearrange("p k (a b) -> p k a b", a=NLC),
                    h1_s[:, oc, :].rearrange("p (a b) -> p a b", a=NLC)
                    .unsqueeze(1).broadcast_to([128, K, NLC, LR]),
                    srep_s[:, oc].unsqueeze(2).broadcast_to([128, K, NLC, LR]))
                dbuf_hoist[oc] = db

            for oc in range(OC):
                if oc in dbuf_hoist:
                    dbuf = dbuf_hoist[oc]
                else:
                    dbuf = dpool.tile([128, K, L], bf16, tag="dbuf")
                    nc.vector.tensor_sub(
                        dbuf[:].rearrange("p k (a b) -> p k a b", a=NLC),
                        h1_s[:, oc, :].rearrange("p (a b) -> p a b", a=NLC)
                        .unsqueeze(1).broadcast_to([128, K, NLC, LR]),
                        srep_s[:, oc].unsqueeze(2).broadcast_to([128, K, NLC, LR]))

                # sigmoids: k < KF fused on ACT; k >= KF via DVE arg + one ACT
                sig = opool.tile([128, K, L], bf16, tag="sig")
                for k in range(KF):
                    nc.scalar.activation(sig[:, k, :], A_sb[:, oc, :], AF.Sigmoid,
                                         bias=cb_s[:, oc, k:k + 1])
                if KS:
                    crep = spool.tile([128, KS, LR], bf16, tag="crep")
                    nc.vector.tensor_copy(
                        crep[:], cb_s[:, oc, KF:].unsqueeze(2)
                        .broadcast_to([128, KS, LR]))
                    arg = dpool.tile([128, KS, L], bf16, tag="arg")
                    nc.vector.tensor_add(
                        arg[:].rearrange("p k (a b) -> p k a b", a=NLC),
                        A_sb[:, oc, :].rearrange("p (a b) -> p a b", a=NLC)
                        .unsqueeze(1).broadcast_to([128, KS, NLC, LR]),
                        crep[:].unsqueeze(2).broadcast_to([128, KS, NLC, LR]))
                    nc.scalar.activation(
                        sig[:, KF:, :].rearrange("p k l -> p (k l)"),
                        arg[:].rearrange("p k l -> p (k l)"), AF.Sigmoid)

                # m = d * sig; out = m + s; DMA per chunk (k-quarters for the
                # last oc so its pipeline drains fast)
                ob = opool.tile([128, K, L], bf16, tag="ob")
                nch = 4 if oc == OC - 1 else 2
                KC = K // nch
                for kc in range(nch):
                    k0, k1 = kc * KC, (kc + 1) * KC
                    kdv = k1 - (GA if k1 == K else 0)
                    nc.vector.tensor_mul(
                        ob[:, k0:k1, :].rearrange("p k l -> p (k l)"),
                        dbuf[:, k0:k1, :].rearrange("p k l -> p (k l)"),
                        sig[:, k0:k1, :].rearrange("p k l -> p (k l)"))
                    nc.vector.tensor_add(
                        ob[:, k0:kdv, :].rearrange("p k (a b) -> p k a b", a=NLC),
                        ob[:, k0:kdv, :].rearrange("p k (a b) -> p k a b", a=NLC),
                        srep_s[:, oc, k0:kdv].unsqueeze(2)
                        .broadcast_to([128, kdv - k0, NLC, LR]))
                    for k in range(kdv, k1):
                        nc.gpsimd.tensor_scalar_add(
                            ob[:, k, :], ob[:, k, :], sF_s[:, oc, k:k + 1])
                    nc.sync.dma_start(out_d[oc][:, k0:k1], ob[:, k0:k1, :])

    nc.compile()
    return nc


def _prep_in_maps(x, s, g, Wq, bq, Wkv, bkv, Wg1, bg1, Wg2, bg2):
    def swz(a):  # [X, H-contract] -> [128, HC, X]: SBUF layout, h on partitions
        aT = np.asarray(a).T  # [H, X]
        return np.ascontiguousarray(
            aT.reshape(HC, 128, -1).transpose(1, 0, 2)).astype(BF16)

    def swz8(a):  # fp8 weight swizzle with x32 prescale
        aT = np.asarray(a).T * W8SCALE
        aT = np.clip(aT, -15.0, 15.0)
        return np.ascontiguousarray(
            aT.reshape(HC, 128, -1).transpose(1, 0, 2)).astype(F8E3)

    def rsh(v):  # (H,) -> [128, H//128] partition-major chunks
        return np.ascontiguousarray(v.reshape(OC, 128).T).astype(np.float32)

    Wk, Wv = Wkv[:H], Wkv[H:]
    W1x, W1y = Wg1[:, :H], Wg1[:, H:]
    W2x, W2y = Wg2[:, :H], Wg2[:, H:]
    w2x8, w2y8 = swz8(W2x), swz8(W2y)  # [128, HC, 768]
    shared = {
        "wq": swz8(Wq), "wk": swz8(Wk), "wv": swz8(Wv),
        "w1x": swz8(W1x), "w1y": swz8(W1y),
    }
    for i in range(3):
        osl = slice(i * 256, (i + 1) * 256)
        shared[f"w2xy{i}"] = np.ascontiguousarray(
            np.stack([w2x8[:, :, osl], w2y8[:, :, osl]], axis=1))
    bstack = np.concatenate(
        [rsh(bq / np.sqrt(H)), rsh(bkv[:H]), rsh(bkv[H:]), rsh(bg1), rsh(bg2)],
        axis=1)
    ident = np.eye(128, dtype=np.float32).astype(BF16)
    in_maps = []
    for b in range(B):
        m = dict(shared)
        # byte-packed small tensors: bstack | xT | gT | sT | ident
        m["acts"] = np.concatenate(
            [bstack.view(np.uint8).reshape(128, -1),
             swz(x[b]).view(np.uint8).reshape(128, -1),
             swz(g[b]).view(np.uint8).reshape(128, -1),
             swz(s[b]).view(np.uint8).reshape(128, -1),
             ident.view(np.uint8).reshape(128, -1)], axis=1)
        # [128, OC, K, LR]: s[k, oc*128+p] replicated along a dummy-l axis
        sr = np.asarray(s[b]).T.reshape(OC, 128, K).transpose(1, 0, 2)
        m["srep"] = np.ascontiguousarray(
            np.broadcast_to(sr[..., None], (128, OC, K, LR))).astype(BF16)
        in_maps.append(m)
    return in_maps


def kernel(**inputs):
    global last_exec_time_ns, last_profile
    from concourse.bass_utils import run_bass_kernel_spmd

    if "nc" not in _CACHE:
        _CACHE["nc"] = _build()
    nc = _CACHE["nc"]

    inputs = {k: np.asarray(v, dtype=np.float32) if np.asarray(v).dtype != np.int32
              else np.asarray(v) for k, v in inputs.items()}
    in_maps = _prep_in_maps(**inputs)

    trace = bool(int(os.environ.get("BASS_KERNEL_TRACE", "0")))
    repeat = int(os.environ.get("BASS_KERNEL_REPEAT", "1"))
    times = []
    for _ in range(repeat):
        res = run_bass_kernel_spmd(nc, in_maps, core_ids=list(range(B)), trace=trace)
        if res.exec_time_ns is not None:
            times.append(res.exec_time_ns)
    if times:
        print(f"exec times: {times}")
        last_exec_time_ns = min(times)
    last_profile = res.profile_json

    out = np.empty((B, L, K, H), dtype=np.float32)
    for b in range(B):
        # per-core result is [OC, 128, K, L] -> [L, K, H]
        r = res.results[b]["out"].astype(np.float32)
        out[b] = np.transpose(r, (3, 2, 0, 1)).reshape(L, K, H)
    return out
